# revision 1
# baseline (speedup 1.0000x reference)
"""Distributed causal attention for TRN2 (8 NeuronCores).

Reference op (per core-external semantics):
    qkv = x @ w_qkv + b_qkv ; split into per-head q,k,v (16 heads, hd=64)
    causal softmax(q k^T / 8) v per head ; concat heads ; out = . @ w_proj + b_proj

Sharding: head-parallel attention (2 heads/core), AllToAll redistribution to
sequence-parallel for the output projection (each core owns S/8 query rows).

All matmuls run in bf16 (fp32 PSUM accumulation); softmax runs without
max-subtraction (scores are bounded: |score| < 4 for this problem's scale),
with denominators obtained via a ones-column appended to V.

kernel(**inputs) takes the FULL fp32 inputs and returns the FULL fp32 output.
"""

import numpy as np
import ml_dtypes

import concourse.bacc as bacc
import concourse.bass as bass
import concourse.tile as tile
from concourse import masks, mybir
from concourse.bass_utils import run_bass_kernel_spmd

N_CORES = 8
D = 1024
H = 16
HD = 64
HPC = H // N_CORES          # heads per core = 2
MQKV = 3 * HPC * HD         # per-core qkv feature cols = 384

BF16 = mybir.dt.bfloat16
F32 = mybir.dt.float32
bf16 = ml_dtypes.bfloat16

# Bumping this changes the compiled executable's signature (a dummy input's
# shape encodes it), forcing a fresh compile + stage. Bump if a crashed run
# leaves a poisoned staged executable behind.
BUILD_SALT = 9



def build(S):
    QB = S // N_CORES        # query rows per core (A2A shard) = 512 for S=4096
    NQ = N_CORES             # number of q blocks == cores
    SKT = S // 128           # total sk tiles
    DIAG = QB // 128         # diagonal sk tiles per q block
    NPROJ = S // 512         # qkv-proj N blocks of 512
    MT = QB // 128           # out-row subtiles of 128
    G = 2                    # sk tiles per exp group

    nc = bacc.Bacc("TRN2", num_devices=N_CORES)

    xT = nc.declare_dram_parameter("xT", [D, S], BF16, isOutput=False)
    wqkv = nc.declare_dram_parameter("wqkv", [D, MQKV], BF16, isOutput=False)
    bqkv = nc.declare_dram_parameter("bqkv", [1, MQKV], BF16, isOutput=False)
    wproj = nc.declare_dram_parameter("wproj", [D, D], BF16, isOutput=False)
    bproj = nc.declare_dram_parameter("bproj", [1, D], BF16, isOutput=False)
    maskp = nc.declare_dram_parameter("mask", [QB, QB], BF16, isOutput=False)
    salt = nc.declare_dram_parameter("salt", [1, BUILD_SALT], F32, isOutput=False)
    out_ext = nc.declare_dram_parameter("out", [QB, D], F32, isOutput=True)

    # q-column windows per head (a window gets its own staging + collective)
    WINDOWS = [[(0, QB)] for _ in range(HPC)]

    a2a_in = [
        [nc.dram_tensor(f"a2a_in{h}_{iw}", [NQ, HD, qw], BF16)
         for iw, (q0, qw) in enumerate(WINDOWS[h])]
        for h in range(HPC)
    ]
    a2a_out = [
        [nc.dram_tensor(f"a2a_out{h}_{iw}", [NQ, HD, qw], BF16)
         for iw, (q0, qw) in enumerate(WINDOWS[h])]
        for h in range(HPC)
    ]
    rden_dram = nc.dram_tensor("rden_dram", [HPC, NQ, QB], F32)

    with tile.TileContext(nc) as tc:
        with (
            tc.tile_pool(name="singles", bufs=1) as singles,
            tc.tile_pool(name="work", bufs=2) as work,
            tc.tile_pool(name="norm", bufs=4) as norm,
            tc.tile_pool(name="ppool", bufs=8) as ppool,
            tc.tile_pool(name="upool", bufs=12) as upool,
            tc.tile_pool(name="ps1", bufs=3, space="PSUM") as ps1,
            tc.tile_pool(name="ps2", bufs=2, space="PSUM") as ps2,
        ):
            # ---- load phase ----
            # small attention-critical loads first; x arrives in (row-chunk,
            # seq-block) pieces so projection N-tile 0 starts after ~1 MB
            w_sb = singles.tile([128, 8, MQKV], BF16)
            nc.sync.dma_start(out=w_sb[:], in_=wqkv.rearrange("(a p) m -> p a m", p=128))
            bq_sb = singles.tile([1, MQKV], BF16)
            nc.sync.dma_start(out=bq_sb[:], in_=bqkv[:])
            mask_sb = singles.tile([128, DIAG, QB], BF16)
            nc.sync.dma_start(out=mask_sb[:], in_=maskp.rearrange("(d p) q -> p d q", p=128))
            ones_sb = singles.tile([1, 512], BF16)
            nc.vector.memset(ones_sb[:], 1.0)
            ident = singles.tile([128, 128], BF16)
            masks.make_identity(nc, ident[:])
            x_sb = singles.tile([128, 8, S], BF16)
            xT_r = xT.rearrange("(a p) s -> p a s", p=128)
            for n in range(NPROJ):
                for a in range(8):
                    nc.sync.dma_start(
                        out=x_sb[:, a, 512 * n:512 * (n + 1)],
                        in_=xT_r[:, a, 512 * n:512 * (n + 1)],
                    )
            wp_sb = singles.tile([128, 8, D], BF16)
            nc.sync.dma_start(out=wp_sb[:], in_=wproj.rearrange("(a p) m -> p a m", p=128))
            bp_sb = singles.tile([1, D], BF16)
            nc.sync.dma_start(out=bp_sb[:], in_=bproj[:])
            salt_sb = singles.tile([1, BUILD_SALT], F32)
            nc.sync.dma_start(out=salt_sb[:], in_=salt[:])

            # ---- qkv^T projection: qkvT[feat, seq] = w^T x (+ b) ----
            qkvT = singles.tile([128, 3, S], BF16)
            for n in range(NPROJ):
                for m in range(3):
                    ps = ps1.tile([128, 512], F32, tag="ps1")
                    for a in range(8):
                        nc.tensor.matmul(
                            ps[:],
                            lhsT=w_sb[:, a, 128 * m:128 * (m + 1)],
                            rhs=x_sb[:, a, 512 * n:512 * (n + 1)],
                            start=(a == 0), stop=False,
                        )
                    nc.tensor.matmul(
                        ps[:],
                        lhsT=bq_sb[:, 128 * m:128 * (m + 1)],
                        rhs=ones_sb[:],
                        start=False, stop=True,
                    )
                    nc.vector.tensor_copy(qkvT[:, m, 512 * n:512 * (n + 1)], ps[:])

            # ---- V natural layout [sk, hd] with ones column appended ----
            v_sb = singles.tile([128, SKT, 2 * (HD + 1)], BF16)
            nc.vector.memset(v_sb[:, :, HD:HD + 1], 1.0)
            nc.vector.memset(v_sb[:, :, 2 * HD + 1:2 * HD + 2], 1.0)
            for t in range(SKT):
                pt = ps2.tile([128, 128], BF16, tag="ps2")
                nc.tensor.transpose(pt[:], qkvT[:, 2, 128 * t:128 * (t + 1)], ident[:])
                nc.vector.tensor_copy(v_sb[:, t, 0:HD], pt[:, 0:HD])
                nc.vector.tensor_copy(v_sb[:, t, HD + 1:2 * HD + 1], pt[:, HD:2 * HD])

            # ---- attention (per head / q-column window / q block), S^T ----
            # Each head's outputs are staged and sent through their own
            # AllToAll, so head 0's collective overlaps head 1's compute.
            for h in range(HPC):
                for iw, (q0, qw) in enumerate(WINDOWS[h]):
                    Gw = max(1, 1024 // qw)  # sk tiles per exp group
                    un_tiles = []
                    den = work.tile([NQ, QB], F32, tag="den")
                    for qb in range(NQ):
                        nk = (qb + 1) * QB // 128  # causal sk tiles
                        p_tiles = []
                        for g0 in range(0, nk, Gw):
                            w = min(Gw, nk - g0)
                            ps = ps1.tile([128, 1024], F32, tag="ps1")
                            for j in range(w):
                                t = g0 + j
                                nc.tensor.matmul(
                                    ps[:, qw * j:qw * (j + 1)],
                                    lhsT=qkvT[HD * h:HD * (h + 1), 1, 128 * t:128 * (t + 1)],
                                    rhs=qkvT[HD * h:HD * (h + 1), 0, QB * qb + q0:QB * qb + q0 + qw],
                                    start=True, stop=True,
                                )
                            pt = ppool.tile([128, 1024], BF16, tag="p")
                            nc.scalar.activation(
                                pt[:, :qw * w], ps[:, :qw * w],
                                mybir.ActivationFunctionType.Exp, scale=0.125,
                            )
                            p_tiles.append(pt)
                        # causal mask on the diagonal tiles (last DIAG sk tiles)
                        for d in range(DIAG):
                            t = nk - DIAG + d
                            g0, j = divmod(t, Gw)
                            sl = slice(qw * j, qw * (j + 1))
                            nc.vector.tensor_mul(
                                p_tiles[g0][:, sl], p_tiles[g0][:, sl],
                                mask_sb[:, d, q0:q0 + qw],
                            )
                        # PV: out^T (64 rows) + denominator (row 64)
                        po = ps2.tile([HD + 1, QB], F32, tag="ps2")
                        for t in range(nk):
                            g0, j = divmod(t, Gw)
                            nc.tensor.matmul(
                                po[:, :qw],
                                lhsT=v_sb[:, t, (HD + 1) * h:(HD + 1) * (h + 1)],
                                rhs=p_tiles[g0][:, qw * j:qw * (j + 1)],
                                start=(t == 0), stop=(t == nk - 1),
                            )
                        un = upool.tile([HD + 1, QB], F32, tag="unorm")
                        nc.vector.tensor_copy(un[:, :qw], po[:, :qw])
                        un_tiles.append(un)
                        # collect this q-block's denominator row right away
                        nc.sync.dma_start(out=den[qb:qb + 1, :qw], in_=un[HD:HD + 1, :qw])

                    # batched reciprocal for this window, then normalize + stage
                    rden = work.tile([NQ, QB], F32, tag="rden")
                    nc.vector.reciprocal(rden[:, :qw], den[:, :qw])
                    for qb in range(NQ):
                        nc.sync.dma_start(
                            out=rden_dram[h, qb, q0:q0 + qw], in_=rden[qb:qb + 1, :qw]
                        )
                    for qb in range(NQ):
                        bc = norm.tile([HD, QB], F32, tag="bcast")
                        src = bass.AP(
                            tensor=rden_dram,
                            offset=(h * NQ + qb) * QB + q0,
                            ap=[[0, HD], [1, qw]],
                        )
                        nc.sync.dma_start(out=bc[:, :qw], in_=src)
                        st = norm.tile([HD, QB], BF16, tag="stage")
                        nc.vector.tensor_mul(st[:, :qw], un_tiles[qb][0:HD, :qw], bc[:, :qw])
                        nc.sync.dma_start(out=a2a_in[h][iw][qb], in_=st[:, :qw])

                    # ---- AllToAll for this (head, window) ----
                    nc.gpsimd.collective_compute(
                        "AllToAll",
                        mybir.AluOpType.bypass,
                        replica_groups=[list(range(N_CORES))],
                        ins=[a2a_in[h][iw][:]],
                        outs=[a2a_out[h][iw][:]],
                    )

            # ---- output projection on local QB rows ----
            # ao is split at the window boundary so the first half's
            # projection can start while the last collective is in flight.
            half = QB // 2 if (QB // 2) % 128 == 0 else QB
            parts = [(0, half)] + ([(half, QB - half)] if half < QB else [])
            ao_tiles = {}
            for pi, (c0, cw) in enumerate(parts):
                ao_tiles[c0] = singles.tile(
                    [128, NQ, cw], BF16, name=f"ao{pi}", tag=f"ao{pi}"
                )
            for h in range(HPC):
                for iw, (q0, qw) in enumerate(WINDOWS[h]):
                    for c0, cw in parts:
                        lo = max(q0, c0)
                        hi = min(q0 + qw, c0 + cw)
                        if lo >= hi:
                            continue
                        nc.sync.dma_start(
                            out=ao_tiles[c0][HD * h:HD * (h + 1), :, lo - c0:hi - c0],
                            in_=a2a_out[h][iw][:, :, lo - q0:hi - q0].rearrange(
                                "g p s -> p g s"
                            ),
                        )
            for m in range(MT):
                c0 = next(c for c, cw in parts if c <= 128 * m < c + cw)
                ao = ao_tiles[c0]
                mo = 128 * m - c0
                ob = work.tile([128, D], F32, tag="osb")
                for nh in range(2):
                    pf = ps1.tile([128, 512], F32, tag="ps1")
                    for g in range(8):
                        nc.tensor.matmul(
                            pf[:],
                            lhsT=ao[:, g, mo:mo + 128],
                            rhs=wp_sb[:, g, 512 * nh:512 * (nh + 1)],
                            start=(g == 0), stop=False,
                        )
                    nc.tensor.matmul(
                        pf[:],
                        lhsT=ones_sb[:, 0:128],
                        rhs=bp_sb[:, 512 * nh:512 * (nh + 1)],
                        start=False, stop=True,
                    )
                    nc.vector.tensor_copy(ob[:, 512 * nh:512 * (nh + 1)], pf[:])
                nc.sync.dma_start(out=out_ext[128 * m:128 * (m + 1), :], in_=ob[:])

    nc.compile()
    return nc


def make_in_maps(S, x, w_qkv, b_qkv, w_proj, b_proj):
    """Host-side sharding: returns per-core input dicts (bf16-cast)."""
    QB = S // N_CORES
    x2 = np.ascontiguousarray(x.reshape(S, D))
    xT = np.ascontiguousarray(x2.T).astype(bf16)
    wproj_b = w_proj.astype(bf16)
    bproj_b = b_proj.reshape(1, D).astype(bf16)
    i, j = np.indices((QB, QB))
    mask = (i <= j).astype(bf16)
    in_maps = []
    for c in range(N_CORES):
        cols = []
        bcols = []
        for part in range(3):  # q, k, v
            for hh in range(HPC):
                h = HPC * c + hh
                lo = part * D + HD * h
                cols.append(w_qkv[:, lo:lo + HD])
                bcols.append(b_qkv[lo:lo + HD])
        w_c = np.concatenate(cols, axis=1).astype(bf16)
        b_c = np.concatenate(bcols).reshape(1, MQKV).astype(bf16)
        in_maps.append({
            "xT": xT,
            "wqkv": np.ascontiguousarray(w_c),
            "bqkv": np.ascontiguousarray(b_c),
            "wproj": wproj_b,
            "bproj": bproj_b,
            "mask": mask,
            "salt": np.zeros((1, BUILD_SALT), np.float32),
        })
    return in_maps


_CACHE = {}


def _get_nc(S):
    if S not in _CACHE:
        _CACHE[S] = build(S)
    return _CACHE[S]


def kernel(x, w_qkv, b_qkv, w_proj, b_proj, trace=False):
    x = np.asarray(x, dtype=np.float32)
    w_qkv = np.asarray(w_qkv, dtype=np.float32)
    b_qkv = np.asarray(b_qkv, dtype=np.float32)
    w_proj = np.asarray(w_proj, dtype=np.float32)
    b_proj = np.asarray(b_proj, dtype=np.float32)
    B, S, _ = x.shape
    nc = _get_nc(S)
    in_maps = make_in_maps(S, x, w_qkv, b_qkv, w_proj, b_proj)
    res = run_bass_kernel_spmd(nc, in_maps, core_ids=list(range(N_CORES)), trace=trace)
    QB = S // N_CORES
    out = np.empty((S, D), dtype=np.float32)
    for c in range(N_CORES):
        out[QB * c:QB * (c + 1)] = res.results[c]["out"]
    if trace:
        kernel.last_exec_time_ns = res.exec_time_ns
        kernel.last_result = res
    return out.reshape(B, S, D)

